# revision 38
# baseline (speedup 1.0000x reference)
"""Trainium2 Bass kernel for CP-decomposed conv2d (nn_CPDConvolution2D).

Reference computation (NCHW, fp32):
  h = conv1x1(x, W1)         [N,64,224,224] -> [N,32,224,224]
  h = depthwise 3x1 vertical (pad 1)
  h = depthwise 1x3 horizontal (pad 1)
  y = conv1x1(h, W4) + bias  -> [N,128,224,224]

Sharding: data-parallel over batch, 2 images per core on 8 cores.

The whole pipeline runs in fp16 (the correctness gate is rel_err<2e-2;
fp16 end-to-end lands ~1e-3): x is downcast on host so loads move half
the bytes, y is stored fp16 and upcast on host, and matmuls stream 1
row/cycle instead of fp32's 4.

Per-core layout: images are processed in 7 strips of HB=32 rows.  A
strip's 32 rows are split over 4 "row groups" of GB=8 rows; partition
band b in [0,4) holds group GARRAY[b]=[0,2,1,3][b] on partitions
[32b, 32b+32).  x is loaded as two overlapping 18-row halves: half0
(partitions 0-63) holds strip rows [h0-1, h0+17) and half1 (64-127)
holds [h0+15, h0+33), so groups (0,2) read the SAME local row index in
their respective halves, as do (1,3).

Stage A folds the VERTICAL depthwise into the 1x1 contraction: with
pre-scaled block-diagonal weights W1v_k (rows 0-63 x cols 0-31 =
diag(wv[:,k]) @ W1^T, rows 64-127 x cols 32-63 likewise) three
accumulating matmuls per 2-row chunk produce the vertically-convolved
h2 directly in PSUM -- one matmul covers two groups at once (M=64,
K=128), halo rows never materialize, and the vertical taps cost zero
vector-engine work.  x halo rows are zeroed at image edges so the
vertical padding falls out automatically.

The horizontal 1x3 runs on the DVE: tap 0 as a tensor_scalar
multiply (element-aligned read, 2x mode) and taps 1-2 as in-place
scalar_tensor_tensor accumulates at 1x -- a plain tensor_scalar on a
2-byte-misaligned fp16 read falls into a ~13x slow path on this HW,
and GPSIMD's Q7 software ops are ~10x below their roofline, so GPSIMD
only does memsets and the half1 load's SWDGE queue.  Stage B (1x1,
K=32, M=128) uses PE row-tiling, each band contracting its own
partition range into its own single-bank PSUM tile; bands run in
pairs so consecutive matmuls alternate disjoint PE row-tiles and each
LDWEIGHTS overlaps the other tile's streaming.  The PSUM->SBUF moves
(stage-A copies on ACT; stage-B bias-moves split 9 ACT / 7 DVE) carry
the bias add fused in.  Two hardware constraints shaped the design:
matmul PSUM outputs must be dense within one 2KB bank (so moves are
2-row granular), and two concurrently-open accumulation groups in one
bank corrupt each other even on disjoint partition ranges (so the two
stage-A pair-groups run sequentially per chunk).
"""
import os
import sys
import types

sys.path.insert(0, '/opt/trn_rl_repo')

import numpy as np

import concourse.bass as bass
import concourse.mybir as mybir
from concourse.tile import TileContext

# ---------------------------------------------------------------------------
# Environment compat: NTFF profile hook (for trace timing) and a sync
# legalizer for this container's walrus build, which accepts at most one
# sem wait and one sem update per instruction while Tile attaches several
# at dependency joins.
# ---------------------------------------------------------------------------


def _install_ntff_hook():
    if "antenv.axon_hooks" in sys.modules:
        return
    try:
        from trn_agent_boot.trn_boot import _ntff_profile_via_ctypes
    except ImportError:
        return
    _hook = _ntff_profile_via_ctypes('/opt/axon/libaxon_pjrt.so')
    m = types.ModuleType("antenv.axon_hooks")
    m.get_axon_ntff_profile_hook = lambda: _hook
    m.set_axon_ntff_profile_hook = lambda h: None
    sys.modules["antenv.axon_hooks"] = m
    from concourse import bass_utils
    bass_utils.upload_artifacts = lambda tmpdir: "local://" + tmpdir


def _legalize_sync(nc):
    """Split multi-wait/multi-update instructions onto same-engine NoOps.

    Engine queues execute in order, so waits hoisted onto NoOps placed
    before an instruction still gate it; an update pushed onto a NoOp
    after a compute instruction fires only once that instruction has
    completed (the documented-safe `op; nop().then_inc(sem)` idiom).
    Moving a DMA's completion update is NOT safe -- assert instead.
    """
    for f in nc.m.functions:
        for bb in f.blocks:
            idx = 0
            while idx < len(bb.instructions):
                inst = bb.instructions[idx]
                si = inst.sync_info
                if si is None:
                    idx += 1
                    continue
                waits = si.on_wait
                if waits is not None and len(waits) > 1:
                    extra = list(waits[:-1])
                    del si.on_wait[:-1]
                    for w in extra:
                        nop = mybir.InstNoOp(
                            name=nc.get_next_instruction_name(),
                            engine=inst.engine, ins=[], outs=[],
                        )
                        nop.sync_info = mybir.SyncInfo(on_wait=[w], on_update=[])
                        nc.register_instruction(nop)
                        bb.instructions.insert(idx, nop)
                        idx += 1
                    si = inst.sync_info
                upds = si.on_update
                if upds is not None and len(upds) > 1:
                    assert not isinstance(
                        inst,
                        (mybir.InstDMACopy, mybir.InstDMA, mybir.InstDmaTransposeAnt),
                    ), f"multi-update on DMA instruction {inst.name}"
                    extra = list(upds[1:])
                    del si.on_update[1:]
                    for u in extra:
                        nop = mybir.InstNoOp(
                            name=nc.get_next_instruction_name(),
                            engine=inst.engine, ins=[], outs=[],
                        )
                        nop.sync_info = mybir.SyncInfo(on_wait=[], on_update=[u])
                        nc.register_instruction(nop)
                        bb.instructions.insert(idx + 1, nop)
                idx += 1


# ---------------------------------------------------------------------------
# Problem shapes (hardcoded per spec)
# ---------------------------------------------------------------------------
N_FULL, S_CH, H_IMG, W_IMG = 16, 64, 224, 224
R_CH, T_CH = 32, 128
N_CORES = 8
N_PER_CORE = N_FULL // N_CORES     # 2 images per core
HB = 32                            # strip height (rows)
GB = HB // 4                       # rows per partition group
N_STRIPS = H_IMG // HB             # 7
FP32 = mybir.dt.float32
FP16 = mybir.dt.float16
# Partition band b (partitions [32b, 32b+32)) holds row group GARRAY[b]:
# the paired stage-A matmuls put the half0 groups (0, 1) on bands 0, 2
# and the half1 groups (2, 3) on bands 1, 3.
GARRAY = (0, 2, 1, 3)
# PSUM rows are padded to 256 fp32 so two 224-wide rows fill one 2KB bank
PR = 256

_CACHE = {}
LAST_EXEC_TIME_NS = None


def _build_nc():
    nc = bass.Bass(target_bir_lowering=False)

    x = nc.dram_tensor("x", [N_PER_CORE, S_CH, H_IMG, W_IMG], FP16,
                       kind="ExternalInput")
    # Vertical-tap-scaled block-diagonal stage-A weights, one per tap k.
    w1v = nc.dram_tensor("w1v", [3, 2 * S_CH, 2 * R_CH], FP16,
                         kind="ExternalInput")
    wh = nc.dram_tensor("wh", [128, 3], FP32, kind="ExternalInput")
    w4s = nc.dram_tensor("w4s", [128, 128], FP16, kind="ExternalInput")
    bias = nc.dram_tensor("bias", [128, 1], FP32, kind="ExternalInput")
    y = nc.dram_tensor("y", [N_PER_CORE, T_CH, H_IMG, W_IMG], FP16,
                       kind="ExternalOutput")

    with TileContext(nc) as tc:
        with (
            tc.tile_pool(name="consts", bufs=1) as consts,
            tc.tile_pool(name="xin", bufs=3) as xin,
            tc.tile_pool(name="mid", bufs=2) as mid,
            tc.tile_pool(name="oout", bufs=3) as oout,
            tc.tile_pool(name="h3pool", bufs=3) as h3pool,
            tc.tile_pool(name="psA", bufs=2, space="PSUM") as psumA,
            tc.tile_pool(name="psB", bufs=6, space="PSUM") as psumB,
        ):
            w1v_t = [consts.tile([2 * S_CH, 2 * R_CH], FP16,
                                 name=f"w1v{k}") for k in range(3)]
            wh_t = consts.tile([128, 3], FP32)
            w4s_t = consts.tile([128, 128], FP16)
            bias_t = consts.tile([128, 1], FP32)
            for k in range(3):
                nc.sync.dma_start(out=w1v_t[k][:], in_=w1v[k, :, :])
            nc.sync.dma_start(out=wh_t[:], in_=wh[:, :])
            nc.sync.dma_start(out=w4s_t[:], in_=w4s[:, :])
            nc.sync.dma_start(out=bias_t[:], in_=bias[:, :])

            # Software-pipelined over strips with a two-strip skew:
            # front(t) = load + stage A + horizontal; back(t) = stage B +
            # bias-moves + store, woven between front(t)'s chunk-steps so
            # the PE FIFO always has ready work.
            N_TOT = N_PER_CORE * N_STRIPS
            live = {}

            def load_x(t):
                n, s = divmod(t, N_STRIPS)
                h0 = s * HB
                # ---- load x strip as two overlapping 18-row halves
                # on partition halves:
                # half0 (parts 0-63):   x rows [h0-1,  h0+17)
                # half1 (parts 64-127): x rows [h0+15, h0+33)
                # half0 rides the sync HWDGE ring, half1 the gpsimd
                # SWDGE queue: partitions 0-63 and 64-127 map to
                # disjoint SDMA-engine sets, so the two 64-partition
                # transfers (each capped at half SBUF-port BW) run
                # concurrently and together use all 16 engines.
                XR = 18
                x_t = xin.tile([128, XR, W_IMG], FP16)
                live[("x", t)] = x_t
                if s == 0:
                    nc.gpsimd.memset(x_t[0:S_CH, 0:1, :], 0.0)
                    nc.sync.dma_start(out=x_t[0:S_CH, 1:XR, :],
                                      in_=x[n, :, 0:XR - 1, :])
                    nc.gpsimd.dma_start(out=x_t[S_CH:128, :, :],
                                        in_=x[n, :, 15:15 + XR, :])
                elif s == N_STRIPS - 1:
                    nc.sync.dma_start(out=x_t[0:S_CH, :, :],
                                      in_=x[n, :, h0 - 1:h0 - 1 + XR, :])
                    nc.gpsimd.dma_start(out=x_t[S_CH:128, 0:XR - 1, :],
                                        in_=x[n, :, h0 + 15:h0 + 15 + XR - 1, :])
                    nc.gpsimd.memset(x_t[S_CH:128, XR - 1:XR, :], 0.0)
                else:
                    nc.sync.dma_start(out=x_t[0:S_CH, :, :],
                                      in_=x[n, :, h0 - 1:h0 - 1 + XR, :])
                    nc.gpsimd.dma_start(out=x_t[S_CH:128, :, :],
                                        in_=x[n, :, h0 + 15:h0 + 15 + XR, :])

            def a_step(t, c):
                # ---- stage A chunk-step: 2 output rows per band ----
                # 6 accumulating matmuls: tap k of pair p contracts x
                # local rows (2c + k .. +2) with W1v_k into psA
                # partitions [64p, 64p+64) -- psA ends up holding the
                # vertically-convolved h2 for rows 2c..2c+2 of every
                # group.  Loop order (k, pair) alternates PE tiles
                # (0,0)/(0,64) between consecutive matmuls so each
                # LDWEIGHTS overlaps the other tile's streaming.
                x_t = live[("x", t)]
                if c == 0:
                    live[("h2p", t)] = mid.tile(
                        [128, GB, W_IMG + 2], FP16, tag="h2p",
                        name=f"h2p_{t}")
                    h2p = live[("h2p", t)]
                    nc.gpsimd.memset(h2p[:, :, 0:1], 0.0)
                    nc.gpsimd.memset(h2p[:, :, W_IMG + 1:W_IMG + 2], 0.0)
                h2p = live[("h2p", t)]
                psA = psumA.tile([128, 2, W_IMG], FP32)
                # pair0's accumulation group must close before pair1's
                # opens: the hardware's lazy-zero state is bank-granular,
                # so two open groups in one bank corrupt each other even
                # on disjoint partition ranges (measured on HW; CoreSim's
                # per-partition model accepts it).
                for pair in range(2):
                    for k in range(3):
                        r0 = 8 * pair + 2 * c + k
                        nc.tensor.matmul(
                            psA[64 * pair:64 * pair + 64, :, :],
                            w1v_t[k][:, :],
                            x_t[:, r0:r0 + 2, :],
                            start=(k == 0), stop=(k == 2),
                            tile_position=(0, 64 * pair),
                        )
                nc.scalar.copy(h2p[:, 2 * c:2 * c + 2, 1:W_IMG + 1],
                               psA[:, :, :])

            def horizontal(t):
                h2p = live.pop(("h2p", t))
                live.pop(("x", t))
                # ---- horizontal 1x3 depthwise on DVE ----
                # Tap 0 reads element-aligned and hits the 2x mode; the
                # shifted taps use scalar_tensor_tensor at 1x (a plain
                # tensor_scalar on a 2-byte-misaligned fp16 read falls
                # into a ~13x slow path on this HW, and GPSIMD's Q7
                # software ops are ~10x slower than their roofline).
                h3 = h3pool.tile([128, GB, W_IMG], FP16, tag="h3")
                # tap 0 reads element-aligned and hits the 2x mode; the
                # shifted taps use scalar_tensor_tensor at 1x (a plain
                # tensor_scalar on a 2-byte-misaligned fp16 read falls
                # into a ~13x slow path on this HW)
                nc.vector.tensor_scalar_mul(
                    h3[:, :, :], h2p[:, :, 0:W_IMG], wh_t[:, 0:1])
                for kh in (1, 2):
                    nc.vector.scalar_tensor_tensor(
                        h3[:, :, :], h2p[:, :, kh:kh + W_IMG],
                        wh_t[:, kh:kh + 1], h3[:, :, :],
                        op0=mybir.AluOpType.mult, op1=mybir.AluOpType.add)
                live[("h3", t)] = h3

            def b_step(t, cc):
                h3 = live[("h3", t)]
                if cc == 0:
                    live[("o", t)] = oout.tile(
                        [T_CH, HB, W_IMG], FP16, tag="o_t",
                        name=f"o_t_{t}")
                o_t = live[("o", t)]
                # ---- stage B chunk-step: 2 rows x 4 bands ----
                # Bands run in pairs (b, b+1) so consecutive matmuls
                # alternate disjoint PE row-tiles (LDWEIGHTS of one
                # overlaps the other's streaming).  One 2-row bias-move
                # per band, 8-on-ACT / 8-on-DVE per strip.
                for bp in range(2):
                    psB0 = psumB.tile([128, 2, W_IMG], FP32, tag="psB",
                                      name="psBe")
                    psB1 = psumB.tile([128, 2, W_IMG], FP32, tag="psB",
                                      name="psBo")
                    ps = (psB0, psB1)
                    for i, b in enumerate((2 * bp, 2 * bp + 1)):
                        nc.tensor.matmul(
                            ps[i][:, :, :],
                            w4s_t[32 * b:32 * b + 32, :],
                            h3[32 * b:32 * b + 32,
                               2 * cc:2 * cc + 2, :],
                            start=True, stop=True,
                            tile_position=(32 * b, 0),
                        )
                    for i, b in enumerate((2 * bp, 2 * bp + 1)):
                        orow = GARRAY[b] * GB + 2 * cc
                        # 9 moves on ACT / 7 on DVE per strip balances
                        # ACT (copies + moves) against DVE (horiz + moves)
                        on_act = bp == 0 or (i == 0 and cc == 0)
                        if on_act:
                            nc.scalar.add(
                                o_t[:, orow:orow + 2, :],
                                ps[i][:, :, :], bias_t[:, 0:1])
                        else:
                            nc.vector.tensor_scalar_add(
                                o_t[:, orow:orow + 2, :],
                                ps[i][:, :, :], bias_t[:, 0:1])

            def b_dma(t):
                n, s = divmod(t, N_STRIPS)
                h0 = s * HB
                o_t = live.pop(("o", t))
                live.pop(("h3", t))
                nc.sync.dma_start(out=y[n, :, h0:h0 + HB, :],
                                    in_=o_t[:, :, :])

            NCA = GB // 2           # 4 stage-A chunk-steps (2 rows each)
            NCB = GB // 2           # 4 stage-B chunk-steps (2 rows each)
            for t in range(N_TOT + 2):
                if t < N_TOT:
                    load_x(t)
                    for c in range(NCA):
                        a_step(t, c)
                        if t >= 2:
                            b_step(t - 2, c)
                    if t >= 2:
                        b_dma(t - 2)
                    horizontal(t)
                else:
                    for cc in range(NCB):
                        b_step(t - 2, cc)
                    b_dma(t - 2)

    _legalize_sync(nc)
    return nc


def _prep_weights(s_to_r_weight, depth_vert_weight, depth_hor_weight,
                  r_to_t_weight, r_to_t_bias):
    w1T = s_to_r_weight[:, :, 0, 0].T.astype(np.float32)        # [64, 32]
    wv1 = depth_vert_weight[:, 0, :, 0].astype(np.float32)      # [32, 3]
    w1v = np.zeros((3, 2 * S_CH, 2 * R_CH), dtype=np.float16)
    for k in range(3):
        blk = (w1T * wv1[None, :, k]).astype(np.float16)        # [64, 32]
        w1v[k, 0:S_CH, 0:R_CH] = blk
        w1v[k, S_CH:2 * S_CH, R_CH:2 * R_CH] = blk
    wh = np.ascontiguousarray(
        np.tile(depth_hor_weight[:, 0, 0, :], (4, 1)).astype(np.float32))
    w4s = np.ascontiguousarray(
        np.tile(r_to_t_weight[:, :, 0, 0].T, (4, 1)).astype(np.float16))
    b = np.ascontiguousarray(
        r_to_t_bias.reshape(T_CH, 1).astype(np.float32))
    return w1v, wh, w4s, b


def kernel(x, s_to_r_weight, depth_vert_weight, depth_hor_weight,
           r_to_t_weight, r_to_t_bias):
    global LAST_EXEC_TIME_NS
    _install_ntff_hook()
    from concourse.bass_utils import run_bass_kernel_spmd

    if "nc" not in _CACHE:
        _CACHE["nc"] = _build_nc()
    nc = _CACHE["nc"]

    x = np.asarray(x, dtype=np.float32).astype(np.float16)
    w1v, wh, w4s, b = _prep_weights(
        np.asarray(s_to_r_weight), np.asarray(depth_vert_weight),
        np.asarray(depth_hor_weight), np.asarray(r_to_t_weight),
        np.asarray(r_to_t_bias))

    in_maps = []
    for i in range(N_CORES):
        in_maps.append({
            "x": np.ascontiguousarray(x[i * N_PER_CORE:(i + 1) * N_PER_CORE]),
            "w1v": w1v, "wh": wh, "w4s": w4s, "bias": b,
        })

    trace = bool(int(os.environ.get("KERNEL_TRACE", "0")))
    res = run_bass_kernel_spmd(nc, in_maps, core_ids=list(range(N_CORES)),
                               trace=trace)
    LAST_EXEC_TIME_NS = res.exec_time_ns

    out = np.empty((N_FULL, T_CH, H_IMG, W_IMG), dtype=np.float32)
    for i in range(N_CORES):
        out[i * N_PER_CORE:(i + 1) * N_PER_CORE] = \
            res.results[i]["y"].astype(np.float32)
    return out


# revision 39
# speedup vs baseline: 1.1256x; 1.1256x over previous
"""Trainium2 Bass kernel for CP-decomposed conv2d (nn_CPDConvolution2D).

Reference computation (NCHW, fp32):
  h = conv1x1(x, W1)         [N,64,224,224] -> [N,32,224,224]
  h = depthwise 3x1 vertical (pad 1)
  h = depthwise 1x3 horizontal (pad 1)
  y = conv1x1(h, W4) + bias  -> [N,128,224,224]

Sharding: data-parallel over batch, 2 images per core on 8 cores.

The whole pipeline runs in fp16 (the correctness gate is rel_err<2e-2;
fp16 end-to-end lands ~1e-3): x is downcast on host so loads move half
the bytes, y is stored fp16 and upcast on host, and matmuls stream 1
row/cycle instead of fp32's 4.

Per-core layout: images are processed in 7 strips of HB=32 rows.  A
strip's 32 rows are split over 4 "row groups" of GB=8 rows; partition
band b in [0,4) holds group GARRAY[b]=[0,2,1,3][b] on partitions
[32b, 32b+32).  x is loaded as two overlapping 18-row halves: half0
(partitions 0-63) holds strip rows [h0-1, h0+17) and half1 (64-127)
holds [h0+15, h0+33), so groups (0,2) read the SAME local row index in
their respective halves, as do (1,3).

Stage A folds the VERTICAL depthwise into the 1x1 contraction: with
pre-scaled block-diagonal weights W1v_k (rows 0-63 x cols 0-31 =
diag(wv[:,k]) @ W1^T, rows 64-127 x cols 32-63 likewise) three
accumulating matmuls per 2-row chunk produce the vertically-convolved
h2 directly in PSUM -- one matmul covers two groups at once (M=64,
K=128), halo rows never materialize, and the vertical taps cost zero
vector-engine work.  x halo rows are zeroed at image edges so the
vertical padding falls out automatically.

The horizontal 1x3 runs on the DVE: tap 0 as a tensor_scalar
multiply (element-aligned read, 2x mode) and taps 1-2 as in-place
scalar_tensor_tensor accumulates at 1x -- a plain tensor_scalar on a
2-byte-misaligned fp16 read falls into a ~13x slow path on this HW,
and GPSIMD's Q7 software ops are ~10x below their roofline, so GPSIMD
only does memsets and the half1 load's SWDGE queue.  Stage B (1x1,
K=32, M=128) uses PE row-tiling, each band contracting its own
partition range into its own single-bank PSUM tile; bands run in
pairs so consecutive matmuls alternate disjoint PE row-tiles and each
LDWEIGHTS overlaps the other tile's streaming.  The PSUM->SBUF moves
(stage-A copies on ACT; stage-B bias-moves split 9 ACT / 7 DVE) carry
the bias add fused in.  Two hardware constraints shaped the design:
matmul PSUM outputs must be dense within one 2KB bank (so moves are
2-row granular), and two concurrently-open accumulation groups in one
bank corrupt each other even on disjoint partition ranges (so the two
stage-A pair-groups run sequentially per chunk).
"""
import os
import sys
import types

sys.path.insert(0, '/opt/trn_rl_repo')

import numpy as np

import concourse.bass as bass
import concourse.mybir as mybir
from concourse.tile import TileContext

# ---------------------------------------------------------------------------
# Environment compat: NTFF profile hook (for trace timing) and a sync
# legalizer for this container's walrus build, which accepts at most one
# sem wait and one sem update per instruction while Tile attaches several
# at dependency joins.
# ---------------------------------------------------------------------------


def _install_ntff_hook():
    if "antenv.axon_hooks" in sys.modules:
        return
    try:
        from trn_agent_boot.trn_boot import _ntff_profile_via_ctypes
    except ImportError:
        return
    _hook = _ntff_profile_via_ctypes('/opt/axon/libaxon_pjrt.so')
    m = types.ModuleType("antenv.axon_hooks")
    m.get_axon_ntff_profile_hook = lambda: _hook
    m.set_axon_ntff_profile_hook = lambda h: None
    sys.modules["antenv.axon_hooks"] = m
    from concourse import bass_utils
    bass_utils.upload_artifacts = lambda tmpdir: "local://" + tmpdir


def _legalize_sync(nc):
    """Split multi-wait/multi-update instructions onto same-engine NoOps.

    Engine queues execute in order, so waits hoisted onto NoOps placed
    before an instruction still gate it; an update pushed onto a NoOp
    after a compute instruction fires only once that instruction has
    completed (the documented-safe `op; nop().then_inc(sem)` idiom).
    Moving a DMA's completion update is NOT safe -- assert instead.
    """
    for f in nc.m.functions:
        for bb in f.blocks:
            idx = 0
            while idx < len(bb.instructions):
                inst = bb.instructions[idx]
                si = inst.sync_info
                if si is None:
                    idx += 1
                    continue
                waits = si.on_wait
                if waits is not None and len(waits) > 1:
                    extra = list(waits[:-1])
                    del si.on_wait[:-1]
                    for w in extra:
                        nop = mybir.InstNoOp(
                            name=nc.get_next_instruction_name(),
                            engine=inst.engine, ins=[], outs=[],
                        )
                        nop.sync_info = mybir.SyncInfo(on_wait=[w], on_update=[])
                        nc.register_instruction(nop)
                        bb.instructions.insert(idx, nop)
                        idx += 1
                    si = inst.sync_info
                upds = si.on_update
                if upds is not None and len(upds) > 1:
                    assert not isinstance(
                        inst,
                        (mybir.InstDMACopy, mybir.InstDMA, mybir.InstDmaTransposeAnt),
                    ), f"multi-update on DMA instruction {inst.name}"
                    extra = list(upds[1:])
                    del si.on_update[1:]
                    for u in extra:
                        nop = mybir.InstNoOp(
                            name=nc.get_next_instruction_name(),
                            engine=inst.engine, ins=[], outs=[],
                        )
                        nop.sync_info = mybir.SyncInfo(on_wait=[], on_update=[u])
                        nc.register_instruction(nop)
                        bb.instructions.insert(idx + 1, nop)
                idx += 1


# ---------------------------------------------------------------------------
# Problem shapes (hardcoded per spec)
# ---------------------------------------------------------------------------
N_FULL, S_CH, H_IMG, W_IMG = 16, 64, 224, 224
R_CH, T_CH = 32, 128
N_CORES = 8
N_PER_CORE = N_FULL // N_CORES     # 2 images per core
HB = 32                            # strip height (rows)
GB = HB // 4                       # rows per partition group
N_STRIPS = H_IMG // HB             # 7
FP32 = mybir.dt.float32
FP16 = mybir.dt.float16
# Partition band b (partitions [32b, 32b+32)) holds row group GARRAY[b]:
# the paired stage-A matmuls put the half0 groups (0, 1) on bands 0, 2
# and the half1 groups (2, 3) on bands 1, 3.
GARRAY = (0, 2, 1, 3)
# PSUM rows are padded to 256 fp32 so two 224-wide rows fill one 2KB bank
PR = 256

_CACHE = {}
LAST_EXEC_TIME_NS = None


def _build_nc():
    nc = bass.Bass(target_bir_lowering=False)

    x = nc.dram_tensor("x", [N_PER_CORE, S_CH, H_IMG, W_IMG], FP16,
                       kind="ExternalInput")
    # Vertical-tap-scaled block-diagonal stage-A weights, one per tap k.
    w1v = nc.dram_tensor("w1v", [3, 2 * S_CH, 2 * R_CH], FP16,
                         kind="ExternalInput")
    wh = nc.dram_tensor("wh", [128, 3], FP32, kind="ExternalInput")
    w4s = nc.dram_tensor("w4s", [128, 128], FP16, kind="ExternalInput")
    bias = nc.dram_tensor("bias", [128, 1], FP32, kind="ExternalInput")
    y = nc.dram_tensor("y", [N_PER_CORE, T_CH, H_IMG, W_IMG], FP16,
                       kind="ExternalOutput")

    with TileContext(nc) as tc:
        with (
            tc.tile_pool(name="consts", bufs=1) as consts,
            tc.tile_pool(name="xin", bufs=3) as xin,
            tc.tile_pool(name="mid", bufs=2) as mid,
            tc.tile_pool(name="oout", bufs=3) as oout,
            tc.tile_pool(name="h3pool", bufs=3) as h3pool,
            tc.tile_pool(name="psA", bufs=2, space="PSUM") as psumA,
            tc.tile_pool(name="psB", bufs=6, space="PSUM") as psumB,
        ):
            w1v_t = [consts.tile([2 * S_CH, 2 * R_CH], FP16,
                                 name=f"w1v{k}") for k in range(3)]
            wh_t = consts.tile([128, 3], FP32)
            w4s_t = consts.tile([128, 128], FP16)
            bias_t = consts.tile([128, 1], FP32)
            for k in range(3):
                nc.sync.dma_start(out=w1v_t[k][:], in_=w1v[k, :, :])
            nc.sync.dma_start(out=wh_t[:], in_=wh[:, :])
            nc.sync.dma_start(out=w4s_t[:], in_=w4s[:, :])
            nc.sync.dma_start(out=bias_t[:], in_=bias[:, :])

            # Software-pipelined over strips with a two-strip skew:
            # front(t) = load + stage A + horizontal; back(t) = stage B +
            # bias-moves + store, woven between front(t)'s chunk-steps so
            # the PE FIFO always has ready work.
            N_TOT = N_PER_CORE * N_STRIPS
            live = {}

            def load_x(t):
                n, s = divmod(t, N_STRIPS)
                h0 = s * HB
                # ---- load x strip as two overlapping 18-row halves
                # on partition halves:
                # half0 (parts 0-63):   x rows [h0-1,  h0+17)
                # half1 (parts 64-127): x rows [h0+15, h0+33)
                # half0 rides the sync HWDGE ring, half1 the gpsimd
                # SWDGE queue: partitions 0-63 and 64-127 map to
                # disjoint SDMA-engine sets, so the two 64-partition
                # transfers (each capped at half SBUF-port BW) run
                # concurrently and together use all 16 engines.
                XR = 18
                x_t = xin.tile([128, XR, W_IMG], FP16)
                live[("x", t)] = x_t
                if s == 0:
                    nc.gpsimd.memset(x_t[0:S_CH, 0:1, :], 0.0)
                    nc.sync.dma_start(out=x_t[0:S_CH, 1:XR, :],
                                      in_=x[n, :, 0:XR - 1, :])
                    nc.gpsimd.dma_start(out=x_t[S_CH:128, :, :],
                                        in_=x[n, :, 15:15 + XR, :])
                elif s == N_STRIPS - 1:
                    nc.sync.dma_start(out=x_t[0:S_CH, :, :],
                                      in_=x[n, :, h0 - 1:h0 - 1 + XR, :])
                    nc.gpsimd.dma_start(out=x_t[S_CH:128, 0:XR - 1, :],
                                        in_=x[n, :, h0 + 15:h0 + 15 + XR - 1, :])
                    nc.gpsimd.memset(x_t[S_CH:128, XR - 1:XR, :], 0.0)
                else:
                    nc.sync.dma_start(out=x_t[0:S_CH, :, :],
                                      in_=x[n, :, h0 - 1:h0 - 1 + XR, :])
                    nc.gpsimd.dma_start(out=x_t[S_CH:128, :, :],
                                        in_=x[n, :, h0 + 15:h0 + 15 + XR, :])

            def a_step(t, c):
                # ---- stage A chunk-step: 2 output rows per band ----
                # 6 accumulating matmuls: tap k of pair p contracts x
                # local rows (2c + k .. +2) with W1v_k into psA
                # partitions [64p, 64p+64) -- psA ends up holding the
                # vertically-convolved h2 for rows 2c..2c+2 of every
                # group.  Loop order (k, pair) alternates PE tiles
                # (0,0)/(0,64) between consecutive matmuls so each
                # LDWEIGHTS overlaps the other tile's streaming.
                x_t = live[("x", t)]
                if c == 0:
                    live[("h2p", t)] = mid.tile(
                        [128, GB, W_IMG + 2], FP16, tag="h2p",
                        name=f"h2p_{t}")
                    h2p = live[("h2p", t)]
                    nc.gpsimd.memset(h2p[:, :, 0:1], 0.0)
                    nc.gpsimd.memset(h2p[:, :, W_IMG + 1:W_IMG + 2], 0.0)
                h2p = live[("h2p", t)]
                psA = psumA.tile([128, 2, W_IMG], FP32)
                # pair0's accumulation group must close before pair1's
                # opens: the hardware's lazy-zero state is bank-granular,
                # so two open groups in one bank corrupt each other even
                # on disjoint partition ranges (measured on HW; CoreSim's
                # per-partition model accepts it).
                for pair in range(2):
                    for k in range(3):
                        r0 = 8 * pair + 2 * c + k
                        nc.tensor.matmul(
                            psA[64 * pair:64 * pair + 64, :, :],
                            w1v_t[k][:, :],
                            x_t[:, r0:r0 + 2, :],
                            start=(k == 0), stop=(k == 2),
                            tile_position=(0, 64 * pair),
                        )
                nc.scalar.copy(h2p[:, 2 * c:2 * c + 2, 1:W_IMG + 1],
                               psA[:, :, :])

            def horizontal(t):
                h2p = live.pop(("h2p", t))
                live.pop(("x", t))
                # ---- horizontal 1x3 depthwise on DVE ----
                # Tap 0 reads element-aligned and hits the 2x mode; the
                # shifted taps use scalar_tensor_tensor at 1x (a plain
                # tensor_scalar on a 2-byte-misaligned fp16 read falls
                # into a ~13x slow path on this HW, and GPSIMD's Q7
                # software ops are ~10x slower than their roofline).
                h3 = h3pool.tile([128, GB, W_IMG], FP16, tag="h3")
                # tap 0 reads element-aligned and hits the 2x mode; the
                # shifted taps use scalar_tensor_tensor at 1x (a plain
                # tensor_scalar on a 2-byte-misaligned fp16 read falls
                # into a ~13x slow path on this HW)
                nc.vector.tensor_scalar_mul(
                    h3[:, :, :], h2p[:, :, 0:W_IMG], wh_t[:, 0:1])
                for kh in (1, 2):
                    nc.vector.scalar_tensor_tensor(
                        h3[:, :, :], h2p[:, :, kh:kh + W_IMG],
                        wh_t[:, kh:kh + 1], h3[:, :, :],
                        op0=mybir.AluOpType.mult, op1=mybir.AluOpType.add)
                live[("h3", t)] = h3

            def b_step(t, cc):
                h3 = live[("h3", t)]
                if cc == 0:
                    live[("o", t)] = oout.tile(
                        [T_CH, HB, W_IMG], FP16, tag="o_t",
                        name=f"o_t_{t}")
                o_t = live[("o", t)]
                # ---- stage B chunk-step: 2 rows x 4 bands ----
                # Bands run in pairs (b, b+1) so consecutive matmuls
                # alternate disjoint PE row-tiles (LDWEIGHTS of one
                # overlaps the other's streaming).  One 2-row bias-move
                # per band, 8-on-ACT / 8-on-DVE per strip.
                for bp in range(2):
                    psB0 = psumB.tile([128, 2, W_IMG], FP32, tag="psB",
                                      name="psBe")
                    psB1 = psumB.tile([128, 2, W_IMG], FP32, tag="psB",
                                      name="psBo")
                    ps = (psB0, psB1)
                    for i, b in enumerate((2 * bp, 2 * bp + 1)):
                        nc.tensor.matmul(
                            ps[i][:, :, :],
                            w4s_t[32 * b:32 * b + 32, :],
                            h3[32 * b:32 * b + 32,
                               2 * cc:2 * cc + 2, :],
                            start=True, stop=True,
                            tile_position=(32 * b, 0),
                        )
                    for i, b in enumerate((2 * bp, 2 * bp + 1)):
                        orow = GARRAY[b] * GB + 2 * cc
                        # 9 moves on ACT / 7 on DVE per strip balances
                        # ACT (copies + moves) against DVE (horiz + moves)
                        on_act = bp == 0 or (i == 0 and cc == 0)
                        if on_act:
                            nc.scalar.add(
                                o_t[:, orow:orow + 2, :],
                                ps[i][:, :, :], bias_t[:, 0:1])
                        else:
                            nc.vector.tensor_scalar_add(
                                o_t[:, orow:orow + 2, :],
                                ps[i][:, :, :], bias_t[:, 0:1])

            def b_dma(t):
                n, s = divmod(t, N_STRIPS)
                h0 = s * HB
                o_t = live.pop(("o", t))
                live.pop(("h3", t))
                # stores ride the scalar HWDGE ring so reads (sync and
                # gpsimd rings) and writes overlap instead of FIFO-ing
                # behind each other on one queue
                nc.scalar.dma_start(out=y[n, :, h0:h0 + HB, :],
                                    in_=o_t[:, :, :])

            NCA = GB // 2           # 4 stage-A chunk-steps (2 rows each)
            NCB = GB // 2           # 4 stage-B chunk-steps (2 rows each)
            for t in range(N_TOT + 2):
                if t < N_TOT:
                    load_x(t)
                    for c in range(NCA):
                        a_step(t, c)
                        if t >= 2:
                            b_step(t - 2, c)
                    if t >= 2:
                        b_dma(t - 2)
                    horizontal(t)
                else:
                    for cc in range(NCB):
                        b_step(t - 2, cc)
                    b_dma(t - 2)

    _legalize_sync(nc)
    return nc


def _prep_weights(s_to_r_weight, depth_vert_weight, depth_hor_weight,
                  r_to_t_weight, r_to_t_bias):
    w1T = s_to_r_weight[:, :, 0, 0].T.astype(np.float32)        # [64, 32]
    wv1 = depth_vert_weight[:, 0, :, 0].astype(np.float32)      # [32, 3]
    w1v = np.zeros((3, 2 * S_CH, 2 * R_CH), dtype=np.float16)
    for k in range(3):
        blk = (w1T * wv1[None, :, k]).astype(np.float16)        # [64, 32]
        w1v[k, 0:S_CH, 0:R_CH] = blk
        w1v[k, S_CH:2 * S_CH, R_CH:2 * R_CH] = blk
    wh = np.ascontiguousarray(
        np.tile(depth_hor_weight[:, 0, 0, :], (4, 1)).astype(np.float32))
    w4s = np.ascontiguousarray(
        np.tile(r_to_t_weight[:, :, 0, 0].T, (4, 1)).astype(np.float16))
    b = np.ascontiguousarray(
        r_to_t_bias.reshape(T_CH, 1).astype(np.float32))
    return w1v, wh, w4s, b


def kernel(x, s_to_r_weight, depth_vert_weight, depth_hor_weight,
           r_to_t_weight, r_to_t_bias):
    global LAST_EXEC_TIME_NS
    _install_ntff_hook()
    from concourse.bass_utils import run_bass_kernel_spmd

    if "nc" not in _CACHE:
        _CACHE["nc"] = _build_nc()
    nc = _CACHE["nc"]

    x = np.asarray(x, dtype=np.float32).astype(np.float16)
    w1v, wh, w4s, b = _prep_weights(
        np.asarray(s_to_r_weight), np.asarray(depth_vert_weight),
        np.asarray(depth_hor_weight), np.asarray(r_to_t_weight),
        np.asarray(r_to_t_bias))

    in_maps = []
    for i in range(N_CORES):
        in_maps.append({
            "x": np.ascontiguousarray(x[i * N_PER_CORE:(i + 1) * N_PER_CORE]),
            "w1v": w1v, "wh": wh, "w4s": w4s, "bias": b,
        })

    trace = bool(int(os.environ.get("KERNEL_TRACE", "0")))
    res = run_bass_kernel_spmd(nc, in_maps, core_ids=list(range(N_CORES)),
                               trace=trace)
    LAST_EXEC_TIME_NS = res.exec_time_ns

    out = np.empty((N_FULL, T_CH, H_IMG, W_IMG), dtype=np.float32)
    for i in range(N_CORES):
        out[i * N_PER_CORE:(i + 1) * N_PER_CORE] = \
            res.results[i]["y"].astype(np.float32)
    return out
